# revision 1
# baseline (speedup 1.0000x reference)
"""Cosine-similarity attention map on 8 Trainium2 NeuronCores.

out[b, i, j] = <x[b,:,i], x[b,:,j]> / (||x[b,:,i]|| * ||x[b,:,j]||)
x: [B=4, C=64, N=4096] fp32  ->  out: [B=4, N=4096, N=4096] fp32

Sharding: data-parallel over B (4 batches) x 2-way row-split of the N x N
output -> 8 cores. Each core receives the full x[b] (for the moving operand
and column norms) plus its 2048-column row slice (for the stationary
operand), normalizes columns on device (y = x * rsqrt(sum_c x^2)), and
computes its [2048, 4096] block of the Gram matrix of y with fp32r matmuls.
"""

import sys

sys.path.insert(0, "/opt/trn_rl_repo")

import numpy as np

import concourse.bass as bass
import concourse.mybir as mybir
import concourse.tile as tile
from concourse import bacc
from concourse.bass_utils import run_bass_kernel_spmd
from concourse.vector_clock import ScopedClock, VectorClock

B, C, N = 4, 64, 4096
NCORES = 8
RB = N * B // NCORES  # 2048 output rows per core
MM_N = 512  # moving free dim per matmul (one PSUM bank of fp32)
MM_M = 128  # output partitions per matmul
NJ = N // MM_N  # 8 column chunks
NT = RB // MM_M  # 16 row tiles per core

F32 = mybir.dt.float32
F32R = mybir.dt.float32r
F16 = mybir.dt.float16


class SplitDrainTileContext(tile.TileContext):
    """Stock TileContext attaches a wait for every pending DMA-queue
    semaphore to a single exit Drain; the walrus build here only allows one
    sync-wait per TPB_CTRL instruction ("Too many sync wait commands").
    Emit one drain per pending logical processor instead."""

    def _drain_and_barrier(self, tick_clock, wait_clock):
        gc = tick_clock.global_clock
        n = len(gc)
        for p in range(n):
            t = gc[p]
            if t <= 0:
                continue
            part = VectorClock([t if q == p else 0 for q in range(n)])
            d = self.nc.sync.drain()
            wait_clock.add_sem_waits(d.ins, ScopedClock({None: part}))

        self.nc.all_engine_barrier()
        assert self.sems is not None
        popped = self.nc._tile_sem_poison_stack.pop()
        assert popped is self._sem_poison
        self.nc.clear_and_free_semaphores(list(self.sems.allocated().values()))
        self.nc.all_engine_barrier()


def _build(use_split_drain=False):
    nc = bacc.Bacc("TRN2", target_bir_lowering=False)
    xf = nc.declare_dram_parameter("xf", [C, N], F32, isOutput=False)
    xr = nc.declare_dram_parameter("xr", [C, RB], F32, isOutput=False)
    out = nc.declare_dram_parameter("out", [RB, N], F32, isOutput=True)

    tc_cls = SplitDrainTileContext if use_split_drain else tile.TileContext
    with tc_cls(nc) as tc:
        with (
            tc.tile_pool(name="persist", bufs=1) as persist,
            tc.tile_pool(name="panels", bufs=4) as panels,
            tc.tile_pool(name="mpsum", bufs=2, space="PSUM") as mpsum,
            tc.tile_pool(name="npsum", bufs=4, space="PSUM") as npsum,
        ):
            # Load inputs, chunked so the norm pipeline starts ASAP.
            XF = persist.tile([C, N], F32)
            XR = persist.tile([C, RB], F32)
            for c0 in range(0, RB, 1024):
                nc.sync.dma_start(
                    out=XR[:, c0 : c0 + 1024], in_=xr[:, c0 : c0 + 1024]
                )
            for c0 in range(0, N, 1024):
                nc.sync.dma_start(
                    out=XF[:, c0 : c0 + 1024], in_=xf[:, c0 : c0 + 1024]
                )

            ones_f = persist.tile([C, 1], F32)
            nc.vector.memset(ones_f, 1.0)
            ones_c = persist.tile([C, 1], F16)  # sumsq reduction lhsT
            nc.vector.tensor_copy(ones_c, ones_f)
            ones_rf = persist.tile([1, C], F32)
            nc.vector.memset(ones_rf, 1.0)
            ones_r = persist.tile([1, C], F16)  # K=1 partition-broadcast lhsT
            nc.vector.tensor_copy(ones_r, ones_rf)

            # Normalize columns: y = x * rsqrt(sum_c x^2), in fp16, in
            # 1024-column chunks. Per chunk: square (DVE) -> sum over C via
            # ones-matmul (PE) -> approx reciprocal from PSUM (DVE) -> sqrt
            # to fp16 (ACT) -> partition-broadcast via K=1 matmul (PE) ->
            # y = x * bcast read from PSUM (DVE).
            CH = 512
            SQR16 = persist.tile([C, RB], F16)
            SQF16 = persist.tile([C, N], F16)
            RS = persist.tile([1, N], F32)
            RN16 = persist.tile([1, N], F16)
            RSr = persist.tile([1, RB], F32)
            RNr16 = persist.tile([1, RB], F16)
            YR = persist.tile([C, RB], F16)
            YF = persist.tile([C, N], F16)

            def norm_chunk(x_src, sq, rs, rn16, y, c0):
                cs = slice(c0, c0 + CH)
                nc.scalar.activation(
                    sq[:, cs], x_src[:, cs], mybir.ActivationFunctionType.Square
                )
                pps = npsum.tile([MM_M, MM_N], F32, tag="pps")
                nc.tensor.matmul(
                    pps[0:1, :], lhsT=ones_c, rhs=sq[:, cs], start=True, stop=True
                )
                nc.vector.reciprocal_approx_fast(rs[:, cs], pps[0:1, :])
                nc.scalar.activation(
                    rn16[:, cs], rs[:, cs], mybir.ActivationFunctionType.Sqrt
                )
                nc.tensor.matmul(
                    pps[0:C, :], lhsT=ones_r, rhs=rn16[:, cs], start=True, stop=True
                )
                nc.vector.tensor_mul(y[:, cs], x_src[:, cs], pps[0:C, :])

            for c0 in range(0, RB, CH):  # row slice first: gates lhsT
                norm_chunk(XR, SQR16, RSr, RNr16, YR, c0)

            # Engines run their queues in order, so emit panel 0's first
            # half right after the column chunks it needs (0..3) — its
            # copies would otherwise queue behind the whole preamble.
            def panel_half(panel, t, hh):
                ts_ = slice(t * MM_M, (t + 1) * MM_M)
                for h in (2 * hh, 2 * hh + 1):
                    ps = mpsum.tile([MM_M, 2 * MM_N], F32, tag="ps")
                    for q in range(2):
                        j = 2 * h + q
                        js = slice(j * MM_N, (j + 1) * MM_N)
                        nc.tensor.matmul(
                            ps[:, q * MM_N : (q + 1) * MM_N],
                            lhsT=YR[:, ts_],
                            rhs=YF[:, js],
                            start=True,
                            stop=True,
                        )
                    hs = slice(h * 1024, (h + 1) * 1024)
                    if h % 2 == 0:
                        nc.vector.tensor_copy(panel[:, hs], ps)
                    else:
                        nc.scalar.copy(out=panel[:, hs], in_=ps)
                nc.sync.dma_start(
                    out=out[ts_, 2048 * hh : 2048 * (hh + 1)],
                    in_=panel[:, 2048 * hh : 2048 * (hh + 1)],
                )

            for c0 in range(0, 4 * CH, CH):
                norm_chunk(XF, SQF16, RS, RN16, YF, c0)
            early = []
            for t in range(3):
                pnl = panels.tile([MM_M, N], F32, tag="panel")
                panel_half(pnl, t, 0)
                early.append(pnl)
            for c0 in range(4 * CH, N, CH):
                norm_chunk(XF, SQF16, RS, RN16, YF, c0)
            for t in range(3):
                panel_half(early[t], t, 1)

            # Gram matrix: out[i, j] = sum_c YR[c, i] * YF[c, j].
            # 4 matmuls fill a 4-bank PSUM tile; plain PSUM->SBUF copies
            # split between DVE (vector) and ACT (scalar); one contiguous
            # 2 MiB DMA per 128-row panel.
            for t in range(3, NT):
                panel = panels.tile([MM_M, N], F32)
                ts_ = slice(t * MM_M, (t + 1) * MM_M)
                for h in range(4):
                    ps = mpsum.tile([MM_M, 2 * MM_N], F32, tag="ps")
                    for q in range(2):
                        j = 2 * h + q
                        js = slice(j * MM_N, (j + 1) * MM_N)
                        qs = slice(q * MM_N, (q + 1) * MM_N)
                        nc.tensor.matmul(
                            ps[:, qs],
                            lhsT=YR[:, ts_],
                            rhs=YF[:, js],
                            start=True,
                            stop=True,
                        )
                    hs = slice(h * 1024, (h + 1) * 1024)
                    if h % 2 == 0:
                        nc.vector.tensor_copy(panel[:, hs], ps)
                    else:
                        nc.scalar.copy(out=panel[:, hs], in_=ps)
                    if h % 2 == 1:
                        nc.sync.dma_start(
                            out=out[ts_, 2048 * (h // 2) : 2048 * (h // 2 + 1)],
                            in_=panel[:, 2048 * (h // 2) : 2048 * (h // 2 + 1)],
                        )

    nc.compile()
    return nc


def _install_profile_hook():
    """This container's antenv lacks axon_hooks, so run_bass_kernel_spmd's
    trace=True path dies on import. Recreate the module and register the
    ctypes NTFF hook that trn_boot would have installed."""
    import sys as _sys
    import types

    if "antenv.axon_hooks" in _sys.modules:
        return
    import antenv

    mod = types.ModuleType("antenv.axon_hooks")
    mod._hook = None

    def set_axon_ntff_profile_hook(h):
        mod._hook = h

    def get_axon_ntff_profile_hook():
        return mod._hook

    mod.set_axon_ntff_profile_hook = set_axon_ntff_profile_hook
    mod.get_axon_ntff_profile_hook = get_axon_ntff_profile_hook
    _sys.modules["antenv.axon_hooks"] = mod
    antenv.axon_hooks = mod

    from trn_agent_boot.trn_boot import _ntff_profile_via_ctypes

    mod.set_axon_ntff_profile_hook(
        _ntff_profile_via_ctypes("/opt/axon/libaxon_pjrt.so")
    )


_nc = None


def _get_nc():
    global _nc
    if _nc is None:
        _nc = _build()
    return _nc


def _run(x, trace=False, trace_cores=None):
    x = np.asarray(x, dtype=np.float32)
    assert x.shape == (B, C, N), x.shape
    core_ids = list(range(NCORES))
    in_maps = []
    for k in core_ids:
        b, r = divmod(k, 2)
        in_maps.append(
            {
                "xf": np.ascontiguousarray(x[b]),
                "xr": np.ascontiguousarray(x[b][:, r * RB : (r + 1) * RB]),
            }
        )
    if trace:
        _install_profile_hook()
    res = run_bass_kernel_spmd(
        _get_nc(), in_maps, core_ids, trace=trace, trace_cores=trace_cores
    )
    out = np.empty((B, N, N), dtype=np.float32)
    for k in core_ids:
        b, r = divmod(k, 2)
        out[b, r * RB : (r + 1) * RB, :] = res.results[k]["out"]
    return out, res


def kernel(x):
    return _run(x)[0]



# revision 2
# speedup vs baseline: 2.1447x; 2.1447x over previous
"""Cosine-similarity attention map on 8 Trainium2 NeuronCores.

out[b, i, j] = <x[b,:,i], x[b,:,j]> / (||x[b,:,i]|| * ||x[b,:,j]||)
x: [B=4, C=64, N=4096] fp32  ->  out: [B=4, N=4096, N=4096] fp32

The output is a symmetric Gram matrix per batch, so each core only
computes a unique half of it (SYRK-style) and the host mirrors the rest
while unsharding. Sharding: 2 cores per batch running the SAME program;
core (b, 0) gets x[b], core (b, 1) gets x[b] with columns reversed.
In its own index space every core computes, for each 128-row tile
a in [0,16): cols [128a, 2048) (triangle part) and cols
[3968-128a, 4096) (anti-diagonal cross part) -- a constant 2176
columns per tile. The identity-core blocks plus the mirrored
reversed-core blocks tile the full matrix exactly once (plus the 16
anti-diagonal blocks twice). Output is written as fp16 (tolerance is
2e-2; fp16 adds ~3e-4) and upcast on the host, so HBM write traffic is
8.9 MiB/core instead of the baseline's 32 MiB/core.
"""

import sys

sys.path.insert(0, "/opt/trn_rl_repo")

import numpy as np

import concourse.bass as bass
import concourse.mybir as mybir
import concourse.tile as tile
from concourse import bacc
from concourse.bass_utils import run_bass_kernel_spmd
from concourse.vector_clock import ScopedClock, VectorClock

B, C, N = 4, 64, 4096
NCORES = 8
NTILES = 16  # 128-row output tiles per core
TW = 2176  # output columns per row tile (constant by construction)
UW = 2 * TW  # two tiles share one 128-partition DMA panel
CH = 512  # norm pipeline chunk (and matmul free-dim / PSUM bank)
NORM_ORDER = [7, 0, 1, 2, 3, 6, 5, 4]  # cols tiles 0/1 need first

F32 = mybir.dt.float32
F16 = mybir.dt.float16


class SplitDrainTileContext(tile.TileContext):
    """Stock TileContext attaches a wait for every pending DMA-queue
    semaphore to a single exit Drain; the walrus build here only allows one
    sync-wait per TPB_CTRL instruction ("Too many sync wait commands").
    Emit one drain per pending logical processor instead."""

    def _drain_and_barrier(self, tick_clock, wait_clock):
        gc = tick_clock.global_clock
        n = len(gc)
        for p in range(n):
            t = gc[p]
            if t <= 0:
                continue
            part = VectorClock([t if q == p else 0 for q in range(n)])
            d = self.nc.sync.drain()
            wait_clock.add_sem_waits(d.ins, ScopedClock({None: part}))

        self.nc.all_engine_barrier()
        assert self.sems is not None
        popped = self.nc._tile_sem_poison_stack.pop()
        assert popped is self._sem_poison
        self.nc.clear_and_free_semaphores(list(self.sems.allocated().values()))
        self.nc.all_engine_barrier()


def _build(use_split_drain=False):
    nc = bacc.Bacc("TRN2", target_bir_lowering=False)
    xf = nc.declare_dram_parameter("xf", [C, N], F32, isOutput=False)
    out = nc.declare_dram_parameter("out", [NTILES // 2 * 128, UW], F16, isOutput=True)

    tc_cls = SplitDrainTileContext if use_split_drain else tile.TileContext
    with tc_cls(nc) as tc:
        with (
            tc.tile_pool(name="persist", bufs=1) as persist,
            tc.tile_pool(name="panels", bufs=3) as panels,
            tc.tile_pool(name="mpsum", bufs=4, space="PSUM") as mpsum,
            tc.tile_pool(name="npsum", bufs=2, space="PSUM") as npsum,
        ):
            XF = persist.tile([C, N], F32)
            for c in NORM_ORDER:
                nc.sync.dma_start(
                    out=XF[:, c * CH : (c + 1) * CH], in_=xf[:, c * CH : (c + 1) * CH]
                )

            ones_f = persist.tile([C, 1], F32)
            nc.vector.memset(ones_f, 1.0)
            ones_c = persist.tile([C, 1], F16)  # sumsq reduction lhsT
            nc.vector.tensor_copy(ones_c, ones_f)
            ones_rf = persist.tile([1, C], F32)
            nc.vector.memset(ones_rf, 1.0)
            ones_r = persist.tile([1, C], F16)  # K=1 partition-broadcast lhsT
            nc.vector.tensor_copy(ones_r, ones_rf)

            # Normalize columns: y = x * rsqrt(sum_c x^2), in fp16, in
            # 512-column chunks. Per chunk: square (ACT) -> sum over C via
            # ones-matmul (PE) -> approx reciprocal from PSUM (DVE) -> sqrt
            # to fp16 (ACT) -> partition-broadcast via K=1 matmul (PE) ->
            # y = x * bcast read from PSUM (DVE).
            SQ = persist.tile([C, N], F16)
            RS = persist.tile([1, N], F32)
            RN16 = persist.tile([1, N], F16)
            YF = persist.tile([C, N], F16)

            for c in NORM_ORDER:
                cs = slice(c * CH, (c + 1) * CH)
                nc.scalar.activation(
                    SQ[:, cs], XF[:, cs], mybir.ActivationFunctionType.Square
                )
                pps = npsum.tile([128, CH], F32, tag="pps")
                nc.tensor.matmul(
                    pps[0:1, :], lhsT=ones_c, rhs=SQ[:, cs], start=True, stop=True
                )
                nc.vector.reciprocal_approx_fast(RS[:, cs], pps[0:1, :])
                nc.scalar.activation(
                    RN16[:, cs], RS[:, cs], mybir.ActivationFunctionType.Sqrt
                )
                nc.tensor.matmul(
                    pps[0:C, :], lhsT=ones_r, rhs=RN16[:, cs], start=True, stop=True
                )
                nc.vector.tensor_mul(YF[:, cs], XF[:, cs], pps[0:C, :])

            # SYRK blocks. Row tile a: lhsT = YF cols [128a, 128a+128);
            # moving cols [128a, 2048) then [3968-128a, 4096). Matmuls in
            # <=512-col chunks into 1-bank PSUM tiles; PSUM->SBUF fp16
            # copies alternate DVE/ACT; one 1.1 MiB DMA per 2-tile panel.
            ncopy = 0
            for u in range(NTILES // 2):
                panel = panels.tile([128, UW], F16, tag="panel")
                for s in range(2):
                    a = 2 * u + s
                    lhsT = YF[:, 128 * a : 128 * a + 128]
                    off = s * TW
                    for start, width in (
                        (128 * a, 2048 - 128 * a),
                        (3968 - 128 * a, 128 + 128 * a),
                    ):
                        done = 0
                        while done < width:
                            csz = min(CH, width - done)
                            ps = mpsum.tile([128, CH], F32, tag="ps")
                            nc.tensor.matmul(
                                ps[:, :csz],
                                lhsT=lhsT,
                                rhs=YF[:, start + done : start + done + csz],
                                start=True,
                                stop=True,
                            )
                            if ncopy % 2 == 0:
                                nc.vector.tensor_copy(
                                    panel[:, off : off + csz], ps[:, :csz]
                                )
                            else:
                                nc.scalar.copy(
                                    out=panel[:, off : off + csz], in_=ps[:, :csz]
                                )
                            ncopy += 1
                            off += csz
                            done += csz
                nc.sync.dma_start(out=out[u * 128 : (u + 1) * 128, :], in_=panel)

    nc.compile()
    return nc


def _install_profile_hook():
    """This container's antenv lacks axon_hooks, so run_bass_kernel_spmd's
    trace=True path dies on import. Recreate the module and register the
    ctypes NTFF hook that trn_boot would have installed."""
    import sys as _sys
    import types

    if "antenv.axon_hooks" in _sys.modules:
        return
    import antenv

    mod = types.ModuleType("antenv.axon_hooks")
    mod._hook = None

    def set_axon_ntff_profile_hook(h):
        mod._hook = h

    def get_axon_ntff_profile_hook():
        return mod._hook

    mod.set_axon_ntff_profile_hook = set_axon_ntff_profile_hook
    mod.get_axon_ntff_profile_hook = get_axon_ntff_profile_hook
    _sys.modules["antenv.axon_hooks"] = mod
    antenv.axon_hooks = mod

    from trn_agent_boot.trn_boot import _ntff_profile_via_ctypes

    mod.set_axon_ntff_profile_hook(
        _ntff_profile_via_ctypes("/opt/axon/libaxon_pjrt.so")
    )


_nc = None


def _get_nc():
    global _nc
    if _nc is None:
        _nc = _build()
    return _nc


# Ordered output blocks (32x32 grid of 128x128) filled by the two cores
# of a batch; the rest is mirrored from the transpose on the host.
_FILLED = np.zeros((32, 32), bool)
for _a in range(16):
    _FILLED[_a, _a:16] = True
    _FILLED[_a, 31 - _a : 32] = True
    _FILLED[31 - _a, 16 : 32 - _a] = True
    _FILLED[31 - _a, 0 : _a + 1] = True
_MISS_I, _MISS_J = np.nonzero(~_FILLED)


def _run(x, trace=False, trace_cores=None):
    x = np.asarray(x, dtype=np.float32)
    assert x.shape == (B, C, N), x.shape
    core_ids = list(range(NCORES))
    in_maps = []
    for k in core_ids:
        b, half = divmod(k, 2)
        xb = x[b] if half == 0 else x[b][:, ::-1]
        in_maps.append({"xf": np.ascontiguousarray(xb)})
    if trace:
        _install_profile_hook()
    res = run_bass_kernel_spmd(
        _get_nc(), in_maps, core_ids, trace=trace, trace_cores=trace_cores
    )
    out = np.empty((B, N, N), dtype=np.float32)
    for k in core_ids:
        b, half = divmod(k, 2)
        O = res.results[k]["out"]
        M = out[b]
        for a in range(NTILES):
            u, s = divmod(a, 2)
            P = O[u * 128 : (u + 1) * 128, s * TW : (s + 1) * TW]
            W1 = 2048 - 128 * a
            if half == 0:
                M[128 * a : 128 * a + 128, 128 * a : 2048] = P[:, :W1]
                M[128 * a : 128 * a + 128, 3968 - 128 * a : 4096] = P[:, W1:]
            else:
                M[3968 - 128 * a : 4096 - 128 * a, 2048 : 4096 - 128 * a] = P[
                    :, :W1
                ][::-1, ::-1]
                M[3968 - 128 * a : 4096 - 128 * a, 0 : 128 * a + 128] = P[:, W1:][
                    ::-1, ::-1
                ]
    for b in range(B):
        Mb = out[b].reshape(32, 128, 32, 128)
        Mb[_MISS_I, :, _MISS_J, :] = Mb[_MISS_J, :, _MISS_I, :].transpose(0, 2, 1)
    return out, res


def kernel(x):
    return _run(x)[0]
